# revision 8
# baseline (speedup 1.0000x reference)
"""LIF spike scan kernel for Trainium2, SPMD over 8 NeuronCores.

Problem: x [B=64, T=8, C=128, H=32, W=32] f32.  Per (b,c,h,w) pixel, scan
over T:  v = tau*u + x_t ; s_t = (v > 1) ; u = v*(v <= 1).  Output spikes
[B, T, C, H, W] f32.

Sharding: pure data-parallel on B (8 per core).  Per core layout: C=128 is
the SBUF partition dim, H*W=1024 the free dim; the T scan runs locally with
the membrane held in SBUF.  No collectives.
"""

import numpy as np

B, T, C, HW = 64, 8, 128, 32 * 32
N_CORES = 8
B_LOC = B // N_CORES
TAU = 0.5
THRESH = 1.0

# Engine/dtype knobs (iterated on during optimization)
SPIKE_ENGINE = "gpsimd"  # "vector" | "gpsimd"
OUT_DT = "uint8"  # "float32" | "uint8" | "bfloat16"

_cache = {}


def _build_nc():
    from concourse import bacc, mybir, tile

    op = mybir.AluOpType
    nc = bacc.Bacc(
        "TRN2", target_bir_lowering=False, debug=False, num_devices=N_CORES
    )
    out_dt = getattr(mybir.dt, OUT_DT)
    x_ext = nc.dram_tensor(
        "x", [B_LOC, T, C, HW], mybir.dt.float32, kind="ExternalInput"
    ).ap()
    # Output stored as [b, c, t*hw] (channel-major) so each partition row is
    # one contiguous 8KB run — DMA descriptors stay >=2KB for full bandwidth.
    # The host unshuffles back to [b, t, c, h, w].
    out_ext = nc.dram_tensor(
        "out", [B_LOC, C, T * HW], out_dt, kind="ExternalOutput"
    ).ap()

    with tile.TileContext(nc) as tc:
        with tc.tile_pool(name="pool", bufs=2) as pool:
            for b in range(B_LOC):
                xall = pool.tile([C, T * HW], mybir.dt.float32, tag="x")
                spk = pool.tile([C, T * HW], out_dt, tag="s")
                mem = pool.tile([C, HW], mybir.dt.float32, tag="m")
                nc.sync.dma_start(
                    out=xall.rearrange("c (t f) -> c t f", t=T),
                    in_=x_ext[b].rearrange("t c f -> c t f"),
                )
                spike_eng = nc.gpsimd if SPIKE_ENGINE == "gpsimd" else nc.vector
                for t in range(T):
                    v = xall[:, t * HW : (t + 1) * HW]
                    if t > 0:
                        # v = tau*mem + x_t   (in place into the x tile)
                        nc.vector.scalar_tensor_tensor(
                            out=v, in0=mem, scalar=TAU, in1=v,
                            op0=op.mult, op1=op.add,
                        )
                    spike_eng.tensor_scalar(
                        out=spk[:, t * HW : (t + 1) * HW], in0=v,
                        scalar1=THRESH, scalar2=None, op0=op.is_gt,
                    )
                    if t < T - 1:
                        # mem = (v <= 1) * v   (hard reset)
                        nc.vector.scalar_tensor_tensor(
                            out=mem, in0=v, scalar=THRESH, in1=v,
                            op0=op.is_le, op1=op.mult,
                        )
                nc.sync.dma_start(out=out_ext[b], in_=spk)
    nc.compile()
    return nc


def _run(x: np.ndarray, trace: bool = False, tmpdir=None):
    from concourse.bass_utils import run_bass_kernel_spmd

    if "nc" not in _cache:
        _cache["nc"] = _build_nc()
    nc = _cache["nc"]
    x4 = np.ascontiguousarray(x.reshape(B, T, C, HW).astype(np.float32, copy=False))
    in_maps = [{"x": x4[i * B_LOC : (i + 1) * B_LOC]} for i in range(N_CORES)]
    res = run_bass_kernel_spmd(
        nc, in_maps, core_ids=list(range(N_CORES)), trace=trace, tmpdir=tmpdir
    )
    _cache["last_results"] = res
    outs = [res.results[i]["out"] for i in range(N_CORES)]
    out = np.concatenate(outs, axis=0)  # [B, C, T*HW]
    if OUT_DT == "bfloat16":
        out = (out.view(np.uint16).astype(np.uint32) << 16).view(np.float32)
    elif out.dtype != np.float32:
        out = out.astype(np.float32)
    # [B, C, T, HW] -> [B, T, C, HW]
    out = out.reshape(B, C, T, HW).transpose(0, 2, 1, 3)
    return np.ascontiguousarray(out).reshape(B, T, C, 32, 32)


def kernel(x: np.ndarray) -> np.ndarray:
    return _run(x, trace=False)


# revision 14
# speedup vs baseline: 5.9413x; 5.9413x over previous
"""LIF spike scan kernel for Trainium2, SPMD over 8 NeuronCores.

Problem: x [B=64, T=8, C=128, H=32, W=32] f32.  Per (b,c,h,w) pixel, scan
over T:  v = tau*u + x_t ; s_t = (v > 1) ; u = v*(v <= 1).  Output spikes
[B, T, C, H, W] f32.

Sharding: pure data-parallel on B (8 per core).  Per core layout: C=128 is
the SBUF partition dim, H*W=1024 the free dim; the T scan runs locally with
the membrane held in SBUF.  No collectives.
"""

import numpy as np

B, T, C, HW = 64, 8, 128, 32 * 32
N_CORES = 8
B_LOC = B // N_CORES
TAU = 0.5
THRESH = 1.0

# Engine/dtype knobs (iterated on during optimization)
SPIKE_ENGINE = "scalar_sign"  # "vector" | "gpsimd" | "scalar_sign"
OUT_DT = "uint8"  # "float32" | "uint8" | "bfloat16"

_cache = {}


def _build_nc():
    from concourse import bacc, mybir, tile

    op = mybir.AluOpType
    nc = bacc.Bacc(
        "TRN2", target_bir_lowering=False, debug=False, num_devices=N_CORES
    )
    out_dt = getattr(mybir.dt, OUT_DT)
    # x is pre-transposed on the host to [b, c, t*hw] so each partition row
    # of the per-b load is one contiguous 32KB run.
    x_ext = nc.dram_tensor(
        "x", [B_LOC, C, T * HW], mybir.dt.float32, kind="ExternalInput"
    ).ap()
    # Output stored as [b, c, t*hw] (channel-major) so each partition row is
    # one contiguous 8KB run — DMA descriptors stay >=2KB for full bandwidth.
    # The host unshuffles back to [b, t, c, h, w].
    out_ext = nc.dram_tensor(
        "out", [B_LOC, C, T * HW], out_dt, kind="ExternalOutput"
    ).ap()

    with tile.TileContext(nc) as tc:
        with tc.tile_pool(name="pool", bufs=2) as pool:
            neg_thresh = pool.tile([C, 1], mybir.dt.float32, tag="bias", bufs=1)
            nc.vector.memset(neg_thresh, -THRESH)
            for b in range(B_LOC):
                xall = pool.tile([C, T * HW], mybir.dt.float32, tag="x")
                spk = pool.tile([C, T * HW], out_dt, tag="s")
                mem = pool.tile([C, HW], mybir.dt.float32, tag="m")
                nc.sync.dma_start(out=xall, in_=x_ext[b])
                for t in range(T):
                    v = xall[:, t * HW : (t + 1) * HW]
                    s = spk[:, t * HW : (t + 1) * HW]
                    if t > 0:
                        # v = tau*mem + x_t   (in place into the x tile)
                        nc.vector.scalar_tensor_tensor(
                            out=v, in0=mem, scalar=TAU, in1=v,
                            op0=op.mult, op1=op.add,
                        )
                    if SPIKE_ENGINE == "scalar_sign":
                        # Sign(v-1) in {-1,0,1}; f32->u8 writeback saturates
                        # the -1 to 0, giving the Heaviside directly.
                        nc.scalar.activation(
                            out=s, in_=v,
                            func=mybir.ActivationFunctionType.Sign,
                            bias=neg_thresh,
                        )
                    else:
                        eng = nc.gpsimd if SPIKE_ENGINE == "gpsimd" else nc.vector
                        eng.tensor_scalar(
                            out=s, in0=v,
                            scalar1=THRESH, scalar2=None, op0=op.is_gt,
                        )
                    if t < T - 1:
                        # mem = (v <= 1) * v   (hard reset)
                        nc.vector.scalar_tensor_tensor(
                            out=mem, in0=v, scalar=THRESH, in1=v,
                            op0=op.is_le, op1=op.mult,
                        )
                nc.sync.dma_start(out=out_ext[b], in_=spk)
    nc.compile()
    return nc


def _run(x: np.ndarray, trace: bool = False, tmpdir=None):
    from concourse.bass_utils import run_bass_kernel_spmd

    if "nc" not in _cache:
        _cache["nc"] = _build_nc()
    nc = _cache["nc"]
    # [B, T, C, HW] -> [B, C, T*HW] so the kernel's per-b loads are contiguous
    x4 = np.ascontiguousarray(
        x.reshape(B, T, C, HW).astype(np.float32, copy=False).transpose(0, 2, 1, 3)
    ).reshape(B, C, T * HW)
    in_maps = [{"x": x4[i * B_LOC : (i + 1) * B_LOC]} for i in range(N_CORES)]
    res = run_bass_kernel_spmd(
        nc, in_maps, core_ids=list(range(N_CORES)), trace=trace, tmpdir=tmpdir
    )
    _cache["last_results"] = res
    outs = [res.results[i]["out"] for i in range(N_CORES)]
    out = np.concatenate(outs, axis=0)  # [B, C, T*HW]
    if OUT_DT == "bfloat16":
        out = (out.view(np.uint16).astype(np.uint32) << 16).view(np.float32)
    elif out.dtype != np.float32:
        out = out.astype(np.float32)
    # [B, C, T, HW] -> [B, T, C, HW]
    out = out.reshape(B, C, T, HW).transpose(0, 2, 1, 3)
    return np.ascontiguousarray(out).reshape(B, T, C, 32, 32)


def kernel(x: np.ndarray) -> np.ndarray:
    return _run(x, trace=False)
